# revision 13
# baseline (speedup 1.0000x reference)
"""Trainium2 Bass kernel for nn_DecoderCrossMSA (Swin-style shifted-window
cross-attention).

Strategy: data-parallel over batch (8 batches -> 8 cores). Host prepares, per
core, feature-major window-ordered activations (token axis permuted so every
8x8 shifted window is a contiguous 64-token run; roll folded into the
permutation). Device pipeline is software-pipelined over 32 window-pairs so
the tensor engine never waits on the softmax chain:

  iter t:  S(t) matmuls -> exp(t) [Act] -> pa(t)=exp*table [DVE]
           dd(t-1) row-sum matmul (block-diag ones) -> rd=1/dd, pn=pa*rd [DVE]
           AV(t-2) matmuls -> psum->SBUF copies [Act+DVE]
           + interleaved input/output projections of neighbouring blocks.

PSUM: st 4 banks (S scores), sm 2 banks (all projection psum + dd, 2-deep
rotation), av_sc/av_sh 1 bank each (compact AV output).
"""

import numpy as np
import ml_dtypes

EMB = 512
HEADS = 16
WS = 8
B = 8
HW = 64
N = HW * HW
EH = EMB // HEADS          # 32
WN = HW // WS              # 8
SHIFT = WS // 2            # 4
NW = WN * WN               # 64 windows
WT = WS * WS               # 64 tokens per window
NCORES = 8
NBLK = 8                   # token blocks per core (512 tokens each)
BLKT = N // NBLK           # 512
NPAIR = 32                 # window pairs per core
MASK_NEG = -30000.0

_bf16 = ml_dtypes.bfloat16


def _build_perm(shift):
    """perm[t] = token index n for window-ordered position t."""
    i, j, w1, w2 = np.meshgrid(
        np.arange(WN), np.arange(WN), np.arange(WS), np.arange(WS), indexing="ij"
    )
    r = (WS * i + w1 + shift) % HW
    c = (WS * j + w2 + shift) % HW
    return (r * HW + c).reshape(-1)


_PERM = _build_perm(SHIFT)
_OPERM = _build_perm(0)

# Reference splits EMB as (e H): head h lives on strided channels e*HEADS+h.
# Permute projection out-channels so head h is the contiguous block h*EH..:
_RHO = np.array([e * HEADS + h for h in range(HEADS) for e in range(EH)])


def _pair_tables(pos_emb):
    """4 pair-type tables [128, 16*64] bf16 of exp(T)^T, head-replicated."""
    idx = np.array([[x, y] for x in range(WS) for y in range(WS)])
    rel = idx[None, :, :] - idx[:, None, :] + WS - 1
    bias = pos_emb[rel[:, :, 0], rel[:, :, 1]].astype(np.float64)

    m = np.zeros((WT, WT), dtype=np.float64)
    s = WS * (WS // 2)
    m[-s:, :-s] = MASK_NEG
    m[:-s, -s:] = MASK_NEG
    r = WT // WS
    col = m.reshape(r, WS, r, WS).transpose(1, 0, 3, 2).reshape(WT, WT)

    t0 = bias
    t1 = bias + m          # row-masked  (i == 7)
    t2 = bias + col        # col-masked  (j == 7)
    t3 = bias + m + col    # corner

    def pair_tab(ta, tb):
        ea = np.exp(ta).T    # [k, q]
        eb = np.exp(tb).T
        stk = np.concatenate([ea, eb], axis=0)           # [128, 64]
        rep = np.tile(stk, (1, HEADS))                    # [128, 16*64]
        return rep.astype(_bf16)

    return np.stack([
        pair_tab(t0, t0),
        pair_tab(t0, t2),
        pair_tab(t1, t1),
        pair_tab(t1, t3),
    ])


def _pair_type(p):
    row = (p // 4) == WN - 1      # window-row i == 7
    colm = (p % 4) == 3           # second window j == 7
    return (2 if row else 0) + (1 if colm else 0)


def _build_bass():
    import concourse.mybir as mybir
    from concourse import bacc
    from concourse.tile import TileContext

    fp32 = mybir.dt.float32
    bf16 = mybir.dt.bfloat16
    AF = mybir.ActivationFunctionType
    ALU = mybir.AluOpType

    nc = bacc.Bacc()

    # ---- DRAM parameters (per-core) ----
    d_in = {}
    for name in ("cw", "sw", "scw", "shw"):
        d_in[name] = nc.declare_dram_parameter(name, [EMB, N], bf16, isOutput=False)
    for name in ("w1t", "w2t", "wsct", "wsht", "wsot", "wshot"):
        d_in[name] = nc.declare_dram_parameter(name, [EMB, EMB], bf16, isOutput=False)
    for name in ("b1r", "b2r", "bsor", "bshor"):
        d_in[name] = nc.declare_dram_parameter(name, [128, 4], fp32, isOutput=False)
    d_in["ptab"] = nc.declare_dram_parameter(
        "ptab", [4, 128, HEADS * WT], bf16, isOutput=False
    )
    d_in["ones2"] = nc.declare_dram_parameter("ones2", [128, 128], bf16, isOutput=False)
    yso = nc.declare_dram_parameter("yso", [EMB, N], fp32, isOutput=True)
    ysho = nc.declare_dram_parameter("ysho", [EMB, N], fp32, isOutput=True)

    with TileContext(nc) as tc:
        with (
            tc.tile_pool(name="const", bufs=1) as cpool,
            tc.tile_pool(name="xg", bufs=2) as xgpool,
            tc.tile_pool(name="cs", bufs=2) as cspool,
            tc.tile_pool(name="v", bufs=8) as vpool,
            tc.tile_pool(name="smx", bufs=2) as smxp,
            tc.tile_pool(name="o", bufs=2) as opool,
            tc.tile_pool(name="y", bufs=4) as ypool,
            tc.tile_pool(name="stps", bufs=1, space="PSUM") as stps,
            tc.tile_pool(name="smps", bufs=2, space="PSUM") as smps,
            tc.tile_pool(name="avps", bufs=1, space="PSUM") as avps,
        ):

            # ---- constants into SBUF (QK-proj(0) deps first) ----
            wts = {}

            def load_w(name, eng):
                wts[name] = []
                for k in range(4):
                    t = cpool.tile([128, EMB], bf16, tag=f"{name}_{k}", name=f"{name}{k}")
                    eng.dma_start(t[:], d_in[name][k * 128:(k + 1) * 128, :])
                    wts[name].append(t)

            bias_t = {}

            def load_b(name, eng):
                t = cpool.tile([128, 4], fp32, tag=name, name=name)
                eng.dma_start(t[:], d_in[name][:])
                bias_t[name] = t

            load_w("w1t", nc.sync)
            load_w("w2t", nc.sync)
            load_b("b1r", nc.sync)
            load_b("b2r", nc.sync)
            load_w("wsct", nc.scalar)
            load_w("wsht", nc.scalar)

            # ---- pipeline state ----
            xg = {}        # blk -> {tensor: [4 k-chunk tiles]}
            cs = {}        # blk -> {"cw": [4], "sw": [4]}
            vt = {}        # pair g -> (vt_sc, vt_sh)
            pa_t = {}      # pair g -> pa tile
            pn_t = {}      # pair g -> pn tile
            osc_t = {}     # blk -> (osc, osh)

            def emit_xg_quarter(b, q):
                """DMA k-chunk q of all 4 input tensors for block b."""
                d = xg.setdefault(b, {})
                c0 = b * BLKT
                for tname, eng in (("cw", nc.sync), ("sw", nc.sync),
                                   ("scw", nc.scalar), ("shw", nc.scalar)):
                    lst = d.setdefault(tname, [None] * 4)
                    t = xgpool.tile([128, BLKT], bf16, tag=f"xg_{tname}_{q}",
                                    name=f"xg{tname}{q}")
                    eng.dma_start(
                        t[:], d_in[tname][q * 128:(q + 1) * 128, c0:c0 + BLKT]
                    )
                    lst[q] = t

            def emit_qk_quarter(b, q):
                """Q/K projection output chunk m=q for block b (2 psum gens)."""
                d = cs.setdefault(b, {})
                for tname, wname, bname in (("cw", "w1t", "b1r"),
                                            ("sw", "w2t", "b2r")):
                    lst = d.setdefault(tname, [None] * 4)
                    ps = smps.tile([128, BLKT], fp32, tag="sm", name="psqk")
                    for k in range(4):
                        nc.tensor.matmul(
                            ps[:],
                            lhsT=wts[wname][k][:, q * 128:(q + 1) * 128],
                            rhs=xg[b][tname][k][:],
                            start=(k == 0), stop=(k == 3),
                        )
                    out = cspool.tile([128, BLKT], bf16, tag=f"cs_{tname}_{q}",
                                      name=f"cs{tname}{q}")
                    nc.scalar.activation(
                        out[:], ps[:], AF.Identity,
                        bias=bias_t[bname][:, q:q + 1],
                    )
                    lst[q] = out

            def emit_v_quarter(b, q):
                """V projections (token-major) for pair g = 4b+q."""
                g = 4 * b + q
                t0 = q * 128
                pair = []
                for tname, wname, vtag in (("scw", "wsct", "vsc"),
                                           ("shw", "wsht", "vsh")):
                    ps = smps.tile([128, EMB], fp32, tag="sm", name="psv")
                    for k in range(4):
                        nc.tensor.matmul(
                            ps[:],
                            lhsT=xg[b][tname][k][:, t0:t0 + 128],
                            rhs=wts[wname][k][:],
                            start=(k == 0), stop=(k == 3),
                        )
                    out = vpool.tile([128, EMB], bf16, tag=vtag, name=vtag)
                    nc.vector.tensor_copy(out[:], ps[:])
                    pair.append(out)
                vt[g] = pair

            def emit_S(g):
                blk, p = g // 4, g % 4
                t0 = p * 128
                cT, sT = cs[blk]["cw"], cs[blk]["sw"]
                st = stps.tile([128, 4 * 512], fp32, tag="st", name="st")
                for h in range(HEADS):
                    m, r = h // 4, (h % 4) * 32
                    s0 = (h % 4) * 512 + (h // 4) * WT
                    for wi in range(2):
                        o0 = t0 + wi * WT
                        nc.tensor.matmul(
                            st[wi * WT:(wi + 1) * WT, s0:s0 + WT],
                            lhsT=sT[m][r:r + 32, o0:o0 + WT],
                            rhs=cT[m][r:r + 32, o0:o0 + WT],
                            start=True, stop=True,
                            tile_position=(r, wi * WT),
                        )
                # exp then multiplicative bias table -> pa (compact layout:
                # free = (h%4)*256 + (h//4)*64 + q)
                st_v = st[:].rearrange("p (b s q) -> p b s q", b=4, s=8, q=WT)[:, :, 0:4, :]
                pe = smxp.tile([128, HEADS * WT], bf16, tag="pe", name="pe")
                pe_v = pe[:].rearrange("p (b s q) -> p b s q", b=4, s=4, q=WT)
                nc.scalar.activation(pe_v, st_v, AF.Exp)
                pa = smxp.tile([128, HEADS * WT], bf16, tag="pa", name="pa")
                nc.vector.tensor_tensor(
                    pa[:], pe[:], ptab_t[_pair_type(g)][:], ALU.mult
                )
                pa_t[g] = pa

            def emit_rowsum(g):
                """dd = per-window column sums of pa(g); pn = pa * (1/dd)."""
                pa = pa_t.pop(g)
                rd = smxp.tile([128, HEADS * WT], fp32, tag="rd", name="rd")
                for half in range(2):
                    fs = slice(half * 512, (half + 1) * 512)
                    dd = smps.tile([128, 512], fp32, tag="sm", name="dd")
                    nc.tensor.matmul(
                        dd[:], lhsT=ones2_t[:], rhs=pa[:, fs],
                        start=True, stop=True,
                    )
                    nc.vector.reciprocal(rd[:, fs], dd[:])
                pn = smxp.tile([128, HEADS * WT], bf16, tag="pn", name="pn", bufs=3)
                nc.vector.tensor_tensor(pn[:], pa[:], rd[:], ALU.mult)
                pn_t[g] = pn

            def emit_AV(g):
                """AV matmuls for pair g into one [128, 1024] PSUM tile:
                free = wi*512 + tensor*256 + m*64 + q. Bank = wi (PE row-group
                -> own bank, the HW wiring rule); sc/sh and h vs h+4 share a
                quadrant so their result streams serialize safely."""
                blk, p = g // 4, g % 4
                t0 = p * 128
                pn = pn_t.pop(g)
                vsc, vsh = vt.pop(g)
                av = avps.tile([128, 1024], fp32, tag="av", name="av")
                for h in range(HEADS):
                    m, r = h // 4, (h % 4) * 32
                    ps0 = (h % 4) * 256 + (h // 4) * WT
                    for wi in range(2):
                        sl = slice(wi * WT, (wi + 1) * WT)
                        for ti, vtile in ((0, vsc), (1, vsh)):
                            f0 = wi * 512 + ti * 256 + m * WT
                            nc.tensor.matmul(
                                av[r:r + 32, f0:f0 + WT],
                                lhsT=vtile[sl, h * 32:(h + 1) * 32],
                                rhs=pn[sl, ps0:ps0 + WT],
                                start=True, stop=True,
                                tile_position=(wi * WT, r),
                            )
                if p == 0:
                    osc = opool.tile([128, 4 * BLKT], bf16, tag="osc", name="osc")
                    osh = opool.tile([128, 4 * BLKT], bf16, tag="osh", name="osh")
                    osc_t[blk] = (osc, osh)
                osc, osh = osc_t[blk]
                # scatter [128, (2 wi, 2 tensor, 4 m, 64 q)] into O tiles
                srcv = av[:].rearrange("p (w t m q) -> p t m w q", w=2, t=2,
                                       m=4, q=WT)
                for ti, o_t, eng in ((0, osc, "scalar"), (1, osh, "vector")):
                    dstv = o_t[:].rearrange("p (m t) -> p m t", m=4)
                    dst = dstv[:, :, t0:t0 + 128].rearrange(
                        "p m (w q) -> p m w q", w=2
                    )
                    if eng == "scalar":
                        nc.scalar.activation(dst, srcv[:, ti], AF.Copy)
                    else:
                        nc.vector.tensor_copy(dst, srcv[:, ti])

            def emit_OP_half(b, half):
                osc, osh = osc_t[b]
                o_t, wname, bname, y_h = (
                    (osc, "wsot", "bsor", yso) if half == 0
                    else (osh, "wshot", "bshor", ysho)
                )
                c0 = b * BLKT
                for mo in range(4):
                    ps = smps.tile([128, BLKT], fp32, tag="sm", name="psop")
                    for k in range(4):
                        nc.tensor.matmul(
                            ps[:],
                            lhsT=wts[wname][k][:, mo * 128:(mo + 1) * 128],
                            rhs=o_t[:, k * BLKT:(k + 1) * BLKT],
                            start=(k == 0), stop=(k == 3),
                        )
                    y_sb = ypool.tile([128, BLKT], fp32, tag="y", name="ysb")
                    nc.scalar.activation(
                        y_sb[:], ps[:], AF.Identity,
                        bias=bias_t[bname][:, mo:mo + 1],
                    )
                    nc.gpsimd.dma_start(
                        y_h[mo * 128:(mo + 1) * 128, c0:c0 + BLKT], y_sb[:]
                    )
                if half == 1:
                    del osc_t[b]

            # ---- prologue: inputs + projections for blocks 0 and 1 ----
            for q in range(4):
                emit_xg_quarter(0, q)
            ptab_t = []
            for i in range(4):
                t = cpool.tile([128, HEADS * WT], bf16, tag=f"ptab{i}", name=f"ptab{i}")
                nc.gpsimd.dma_start(t[:], d_in["ptab"][i])
                ptab_t.append(t)
            ones2_t = cpool.tile([128, 128], bf16, tag="ones2", name="ones2")
            nc.gpsimd.dma_start(ones2_t[:], d_in["ones2"][:])
            load_w("wsot", nc.gpsimd)
            load_w("wshot", nc.gpsimd)
            load_b("bsor", nc.gpsimd)
            load_b("bshor", nc.gpsimd)
            for q in range(4):
                emit_xg_quarter(1, q)
            for q in range(4):
                emit_qk_quarter(0, q)
                emit_v_quarter(0, q)

            # ---- software-pipelined main loop ----
            for t in range(36):
                bn_x = t // 4 + 2      # block whose inputs we DMA
                bn_p = t // 4 + 1      # block whose projections we compute
                q = t % 4
                if t < 32:
                    emit_S(t)
                if 1 <= t <= 32:
                    emit_rowsum(t - 1)
                if 2 <= t <= 33:
                    emit_AV(t - 2)
                if bn_x < NBLK:
                    emit_xg_quarter(bn_x, q)
                if bn_p < NBLK:
                    emit_qk_quarter(bn_p, q)
                    emit_v_quarter(bn_p, q)
                tb = t - 6
                if tb >= 0 and tb % 4 in (0, 1) and tb // 4 < NBLK:
                    emit_OP_half(tb // 4, tb % 4)

    nc.compile()
    return nc


_NC_CACHE = {}
LAST_RESULT = None


def make_in_maps(content, style, scale, shift, W1, b1, W2, b2, Wsc, bsc,
                 Wsh, bsh, Wso, bso, Wsho, bsho, pos_emb):
    inv = 1.0 / np.sqrt(EMB / HEADS)
    f32 = np.float32

    w1t = (np.asarray(W1, f32)[_RHO].T * inv).astype(_bf16)  # [e_in, e_out], scaled
    w2t = np.asarray(W2, f32)[_RHO].T.astype(_bf16)
    wsct = np.asarray(Wsc, f32)[_RHO].T.astype(_bf16)
    wsht = np.asarray(Wsh, f32)[_RHO].T.astype(_bf16)
    wsot = np.asarray(Wso, f32).T.astype(_bf16)
    wshot = np.asarray(Wsho, f32).T.astype(_bf16)
    b1r = (np.asarray(b1, f32)[_RHO] * inv).reshape(4, 128).T.copy()
    b2r = np.asarray(b2, f32)[_RHO].reshape(4, 128).T.copy()
    bso2 = np.asarray(Wso, f32) @ np.asarray(bsc, f32)[_RHO] + np.asarray(bso, f32)
    bsho2 = (np.asarray(Wsho, f32) @ np.asarray(bsh, f32)[_RHO]
             + np.asarray(bsho, f32))
    bsor = bso2.reshape(4, 128).T.copy()
    bshor = bsho2.reshape(4, 128).T.copy()
    ptab = _pair_tables(np.asarray(pos_emb, f32))
    ones2 = np.zeros((128, 128), dtype=_bf16)
    ones2[:64, :64] = 1
    ones2[64:, 64:] = 1

    common = dict(
        w1t=w1t, w2t=w2t, wsct=wsct, wsht=wsht, wsot=wsot, wshot=wshot,
        b1r=b1r, b2r=b2r, bsor=bsor, bshor=bshor, ptab=ptab, ones2=ones2,
    )
    in_maps = []
    for b in range(NCORES):
        m = dict(common)
        for name, full in (("cw", content), ("sw", style),
                           ("scw", scale), ("shw", shift)):
            x = np.asarray(full[b], f32)[_PERM]           # [N, EMB] window order
            m[name] = np.ascontiguousarray(x.T).astype(_bf16)
        in_maps.append(m)
    return in_maps


def kernel(**inputs):
    global LAST_RESULT
    from concourse.bass_utils import run_bass_kernel_spmd

    in_maps = make_in_maps(**inputs)

    if "nc" not in _NC_CACHE:
        _NC_CACHE["nc"] = _build_bass()
    res = run_bass_kernel_spmd(_NC_CACHE["nc"], in_maps, list(range(NCORES)))
    LAST_RESULT = res

    out_sc = np.empty((B, N, EMB), np.float32)
    out_sh = np.empty((B, N, EMB), np.float32)
    for b in range(NCORES):
        out_sc[b][_OPERM] = res.results[b]["yso"].T
        out_sh[b][_OPERM] = res.results[b]["ysho"].T
    return out_sc, out_sh


# revision 14
# speedup vs baseline: 1.0930x; 1.0930x over previous
"""Trainium2 Bass kernel for nn_DecoderCrossMSA (Swin-style shifted-window
cross-attention).

Strategy: data-parallel over batch (8 batches -> 8 cores). Host prepares, per
core, feature-major window-ordered activations (token axis permuted so every
8x8 shifted window is a contiguous 64-token run; roll folded into the
permutation). Device pipeline is software-pipelined over 32 window-pairs so
the tensor engine never waits on the softmax chain:

  iter t:  S(t) matmuls -> exp(t) [Act] -> pa(t)=exp*table [DVE]
           dd(t-1) row-sum matmul (block-diag ones) -> rd=1/dd, pn=pa*rd [DVE]
           AV(t-2) matmuls -> psum->SBUF copies [Act+DVE]
           + interleaved input/output projections of neighbouring blocks.

PSUM: st 4 banks (S scores), sm 2 banks (all projection psum + dd, 2-deep
rotation), av_sc/av_sh 1 bank each (compact AV output).
"""

import numpy as np
import ml_dtypes

EMB = 512
HEADS = 16
WS = 8
B = 8
HW = 64
N = HW * HW
EH = EMB // HEADS          # 32
WN = HW // WS              # 8
SHIFT = WS // 2            # 4
NW = WN * WN               # 64 windows
WT = WS * WS               # 64 tokens per window
NCORES = 8
NBLK = 8                   # token blocks per core (512 tokens each)
BLKT = N // NBLK           # 512
NPAIR = 32                 # window pairs per core
MASK_NEG = -30000.0

_bf16 = ml_dtypes.bfloat16


def _build_perm(shift):
    """perm[t] = token index n for window-ordered position t."""
    i, j, w1, w2 = np.meshgrid(
        np.arange(WN), np.arange(WN), np.arange(WS), np.arange(WS), indexing="ij"
    )
    r = (WS * i + w1 + shift) % HW
    c = (WS * j + w2 + shift) % HW
    return (r * HW + c).reshape(-1)


_PERM = _build_perm(SHIFT)
_OPERM = _build_perm(0)

# Reference splits EMB as (e H): head h lives on strided channels e*HEADS+h.
# Permute projection out-channels so head h is the contiguous block h*EH..:
_RHO = np.array([e * HEADS + h for h in range(HEADS) for e in range(EH)])


def _pair_tables(pos_emb):
    """4 pair-type tables [128, 16*64] bf16 of exp(T)^T, head-replicated."""
    idx = np.array([[x, y] for x in range(WS) for y in range(WS)])
    rel = idx[None, :, :] - idx[:, None, :] + WS - 1
    bias = pos_emb[rel[:, :, 0], rel[:, :, 1]].astype(np.float64)

    m = np.zeros((WT, WT), dtype=np.float64)
    s = WS * (WS // 2)
    m[-s:, :-s] = MASK_NEG
    m[:-s, -s:] = MASK_NEG
    r = WT // WS
    col = m.reshape(r, WS, r, WS).transpose(1, 0, 3, 2).reshape(WT, WT)

    t0 = bias
    t1 = bias + m          # row-masked  (i == 7)
    t2 = bias + col        # col-masked  (j == 7)
    t3 = bias + m + col    # corner

    def pair_tab(ta, tb):
        ea = np.exp(ta).T    # [k, q]
        eb = np.exp(tb).T
        stk = np.concatenate([ea, eb], axis=0)           # [128, 64]
        rep = np.tile(stk, (1, HEADS))                    # [128, 16*64]
        return rep.astype(_bf16)

    return np.stack([
        pair_tab(t0, t0),
        pair_tab(t0, t2),
        pair_tab(t1, t1),
        pair_tab(t1, t3),
    ])


def _pair_type(p):
    row = (p // 4) == WN - 1      # window-row i == 7
    colm = (p % 4) == 3           # second window j == 7
    return (2 if row else 0) + (1 if colm else 0)


def _build_bass():
    import concourse.mybir as mybir
    from concourse import bacc
    from concourse.tile import TileContext

    fp32 = mybir.dt.float32
    bf16 = mybir.dt.bfloat16
    AF = mybir.ActivationFunctionType
    ALU = mybir.AluOpType

    nc = bacc.Bacc()

    # ---- DRAM parameters (per-core) ----
    d_in = {}
    for name in ("cw", "sw", "scw", "shw"):
        d_in[name] = nc.declare_dram_parameter(name, [EMB, N], bf16, isOutput=False)
    for name in ("w1t", "w2t", "wsct", "wsht", "wsot", "wshot"):
        d_in[name] = nc.declare_dram_parameter(name, [EMB, EMB], bf16, isOutput=False)
    for name in ("b1r", "b2r", "bsor", "bshor"):
        d_in[name] = nc.declare_dram_parameter(name, [128, 4], fp32, isOutput=False)
    d_in["ptab"] = nc.declare_dram_parameter(
        "ptab", [4, 128, HEADS * WT], bf16, isOutput=False
    )
    d_in["ones2"] = nc.declare_dram_parameter("ones2", [128, 128], bf16, isOutput=False)
    yso = nc.declare_dram_parameter("yso", [EMB, N], fp32, isOutput=True)
    ysho = nc.declare_dram_parameter("ysho", [EMB, N], fp32, isOutput=True)

    with TileContext(nc) as tc:
        with (
            tc.tile_pool(name="const", bufs=1) as cpool,
            tc.tile_pool(name="xg", bufs=2) as xgpool,
            tc.tile_pool(name="cs", bufs=2) as cspool,
            tc.tile_pool(name="v", bufs=8) as vpool,
            tc.tile_pool(name="smx", bufs=2) as smxp,
            tc.tile_pool(name="o", bufs=2) as opool,
            tc.tile_pool(name="y", bufs=4) as ypool,
            tc.tile_pool(name="stps", bufs=1, space="PSUM") as stps,
            tc.tile_pool(name="smps", bufs=2, space="PSUM") as smps,
            tc.tile_pool(name="avps", bufs=1, space="PSUM") as avps,
        ):

            # ---- constants into SBUF (QK-proj(0) deps first) ----
            wts = {}

            def load_w(name, eng):
                wts[name] = []
                for k in range(4):
                    t = cpool.tile([128, EMB], bf16, tag=f"{name}_{k}", name=f"{name}{k}")
                    eng.dma_start(t[:], d_in[name][k * 128:(k + 1) * 128, :])
                    wts[name].append(t)

            bias_t = {}

            def load_b(name, eng):
                t = cpool.tile([128, 4], fp32, tag=name, name=name)
                eng.dma_start(t[:], d_in[name][:])
                bias_t[name] = t

            load_w("w1t", nc.sync)
            load_w("w2t", nc.sync)
            load_b("b1r", nc.sync)
            load_b("b2r", nc.sync)
            load_w("wsct", nc.sync)
            load_w("wsht", nc.sync)

            # ---- pipeline state ----
            xg = {}        # blk -> {tensor: [4 k-chunk tiles]}
            cs = {}        # blk -> {"cw": [4], "sw": [4]}
            vt = {}        # pair g -> (vt_sc, vt_sh)
            pa_t = {}      # pair g -> pa tile
            pn_t = {}      # pair g -> pn tile
            osc_t = {}     # blk -> (osc, osh)

            def emit_xg_quarter(b, q):
                """DMA k-chunk q of all 4 input tensors for block b."""
                d = xg.setdefault(b, {})
                c0 = b * BLKT
                for tname, eng in (("cw", nc.sync), ("sw", nc.sync),
                                   ("scw", nc.sync), ("shw", nc.sync)):
                    lst = d.setdefault(tname, [None] * 4)
                    t = xgpool.tile([128, BLKT], bf16, tag=f"xg_{tname}_{q}",
                                    name=f"xg{tname}{q}")
                    eng.dma_start(
                        t[:], d_in[tname][q * 128:(q + 1) * 128, c0:c0 + BLKT]
                    )
                    lst[q] = t

            def emit_qk_quarter(b, q):
                """Q/K projection output chunk m=q for block b (2 psum gens)."""
                d = cs.setdefault(b, {})
                for tname, wname, bname in (("cw", "w1t", "b1r"),
                                            ("sw", "w2t", "b2r")):
                    lst = d.setdefault(tname, [None] * 4)
                    ps = smps.tile([128, BLKT], fp32, tag="sm", name="psqk")
                    for k in range(4):
                        nc.tensor.matmul(
                            ps[:],
                            lhsT=wts[wname][k][:, q * 128:(q + 1) * 128],
                            rhs=xg[b][tname][k][:],
                            start=(k == 0), stop=(k == 3),
                        )
                    out = cspool.tile([128, BLKT], bf16, tag=f"cs_{tname}_{q}",
                                      name=f"cs{tname}{q}")
                    nc.scalar.activation(
                        out[:], ps[:], AF.Identity,
                        bias=bias_t[bname][:, q:q + 1],
                    )
                    lst[q] = out

            def emit_v_quarter(b, q):
                """V projections (token-major) for pair g = 4b+q."""
                g = 4 * b + q
                t0 = q * 128
                pair = []
                for tname, wname, vtag in (("scw", "wsct", "vsc"),
                                           ("shw", "wsht", "vsh")):
                    ps = smps.tile([128, EMB], fp32, tag="sm", name="psv")
                    for k in range(4):
                        nc.tensor.matmul(
                            ps[:],
                            lhsT=xg[b][tname][k][:, t0:t0 + 128],
                            rhs=wts[wname][k][:],
                            start=(k == 0), stop=(k == 3),
                        )
                    out = vpool.tile([128, EMB], bf16, tag=vtag, name=vtag)
                    nc.vector.tensor_copy(out[:], ps[:])
                    pair.append(out)
                vt[g] = pair

            def emit_S(g):
                blk, p = g // 4, g % 4
                t0 = p * 128
                cT, sT = cs[blk]["cw"], cs[blk]["sw"]
                st = stps.tile([128, 4 * 512], fp32, tag="st", name="st")
                for h in range(HEADS):
                    m, r = h // 4, (h % 4) * 32
                    s0 = (h % 4) * 512 + (h // 4) * WT
                    for wi in range(2):
                        o0 = t0 + wi * WT
                        nc.tensor.matmul(
                            st[wi * WT:(wi + 1) * WT, s0:s0 + WT],
                            lhsT=sT[m][r:r + 32, o0:o0 + WT],
                            rhs=cT[m][r:r + 32, o0:o0 + WT],
                            start=True, stop=True,
                            tile_position=(r, wi * WT),
                        )
                # exp then multiplicative bias table -> pa (compact layout:
                # free = (h%4)*256 + (h//4)*64 + q)
                st_v = st[:].rearrange("p (b s q) -> p b s q", b=4, s=8, q=WT)[:, :, 0:4, :]
                pe = smxp.tile([128, HEADS * WT], bf16, tag="pe", name="pe")
                pe_v = pe[:].rearrange("p (b s q) -> p b s q", b=4, s=4, q=WT)
                nc.scalar.activation(pe_v, st_v, AF.Exp)
                pa = smxp.tile([128, HEADS * WT], bf16, tag="pa", name="pa")
                nc.vector.tensor_tensor(
                    pa[:], pe[:], ptab_t[_pair_type(g)][:], ALU.mult
                )
                pa_t[g] = pa

            def emit_rowsum(g):
                """dd = per-window column sums of pa(g); pn = pa * (1/dd)."""
                pa = pa_t.pop(g)
                rd = smxp.tile([128, HEADS * WT], fp32, tag="rd", name="rd")
                for half in range(2):
                    fs = slice(half * 512, (half + 1) * 512)
                    dd = smps.tile([128, 512], fp32, tag="sm", name="dd")
                    nc.tensor.matmul(
                        dd[:], lhsT=ones2_t[:], rhs=pa[:, fs],
                        start=True, stop=True,
                    )
                    nc.vector.reciprocal(rd[:, fs], dd[:])
                pn = smxp.tile([128, HEADS * WT], bf16, tag="pn", name="pn", bufs=3)
                nc.vector.tensor_tensor(pn[:], pa[:], rd[:], ALU.mult)
                pn_t[g] = pn

            def emit_AV(g):
                """AV matmuls for pair g into one [128, 1024] PSUM tile:
                free = wi*512 + tensor*256 + m*64 + q. Bank = wi (PE row-group
                -> own bank, the HW wiring rule); sc/sh and h vs h+4 share a
                quadrant so their result streams serialize safely."""
                blk, p = g // 4, g % 4
                t0 = p * 128
                pn = pn_t.pop(g)
                vsc, vsh = vt.pop(g)
                av = avps.tile([128, 1024], fp32, tag="av", name="av")
                for h in range(HEADS):
                    m, r = h // 4, (h % 4) * 32
                    ps0 = (h % 4) * 256 + (h // 4) * WT
                    for wi in range(2):
                        sl = slice(wi * WT, (wi + 1) * WT)
                        for ti, vtile in ((0, vsc), (1, vsh)):
                            f0 = wi * 512 + ti * 256 + m * WT
                            nc.tensor.matmul(
                                av[r:r + 32, f0:f0 + WT],
                                lhsT=vtile[sl, h * 32:(h + 1) * 32],
                                rhs=pn[sl, ps0:ps0 + WT],
                                start=True, stop=True,
                                tile_position=(wi * WT, r),
                            )
                if p == 0:
                    osc = opool.tile([128, 4 * BLKT], bf16, tag="osc", name="osc")
                    osh = opool.tile([128, 4 * BLKT], bf16, tag="osh", name="osh")
                    osc_t[blk] = (osc, osh)
                osc, osh = osc_t[blk]
                # scatter [128, (2 wi, 2 tensor, 4 m, 64 q)] into O tiles
                srcv = av[:].rearrange("p (w t m q) -> p t m w q", w=2, t=2,
                                       m=4, q=WT)
                for ti, o_t, eng in ((0, osc, "scalar"), (1, osh, "vector")):
                    dstv = o_t[:].rearrange("p (m t) -> p m t", m=4)
                    dst = dstv[:, :, t0:t0 + 128].rearrange(
                        "p m (w q) -> p m w q", w=2
                    )
                    if eng == "scalar":
                        nc.scalar.activation(dst, srcv[:, ti], AF.Copy)
                    else:
                        nc.vector.tensor_copy(dst, srcv[:, ti])

            def emit_OP_half(b, half):
                osc, osh = osc_t[b]
                o_t, wname, bname, y_h = (
                    (osc, "wsot", "bsor", yso) if half == 0
                    else (osh, "wshot", "bshor", ysho)
                )
                c0 = b * BLKT
                for mo in range(4):
                    ps = smps.tile([128, BLKT], fp32, tag="sm", name="psop")
                    for k in range(4):
                        nc.tensor.matmul(
                            ps[:],
                            lhsT=wts[wname][k][:, mo * 128:(mo + 1) * 128],
                            rhs=o_t[:, k * BLKT:(k + 1) * BLKT],
                            start=(k == 0), stop=(k == 3),
                        )
                    y_sb = ypool.tile([128, BLKT], fp32, tag="y", name="ysb")
                    nc.scalar.activation(
                        y_sb[:], ps[:], AF.Identity,
                        bias=bias_t[bname][:, mo:mo + 1],
                    )
                    nc.gpsimd.dma_start(
                        y_h[mo * 128:(mo + 1) * 128, c0:c0 + BLKT], y_sb[:]
                    )
                if half == 1:
                    del osc_t[b]

            # ---- prologue: inputs + projections for blocks 0 and 1 ----
            for q in range(4):
                emit_xg_quarter(0, q)
            ptab_t = []
            for i in range(4):
                t = cpool.tile([128, HEADS * WT], bf16, tag=f"ptab{i}", name=f"ptab{i}")
                nc.gpsimd.dma_start(t[:], d_in["ptab"][i])
                ptab_t.append(t)
            ones2_t = cpool.tile([128, 128], bf16, tag="ones2", name="ones2")
            nc.gpsimd.dma_start(ones2_t[:], d_in["ones2"][:])
            load_w("wsot", nc.gpsimd)
            load_w("wshot", nc.gpsimd)
            load_b("bsor", nc.gpsimd)
            load_b("bshor", nc.gpsimd)
            for q in range(4):
                emit_xg_quarter(1, q)
            for q in range(4):
                emit_qk_quarter(0, q)
                emit_v_quarter(0, q)

            # ---- software-pipelined main loop ----
            for t in range(36):
                bn_x = t // 4 + 2      # block whose inputs we DMA
                bn_p = t // 4 + 1      # block whose projections we compute
                q = t % 4
                if t < 32:
                    emit_S(t)
                if 1 <= t <= 32:
                    emit_rowsum(t - 1)
                if 2 <= t <= 33:
                    emit_AV(t - 2)
                if bn_x < NBLK:
                    emit_xg_quarter(bn_x, q)
                if bn_p < NBLK:
                    emit_qk_quarter(bn_p, q)
                    emit_v_quarter(bn_p, q)
                tb = t - 6
                if tb >= 0 and tb % 4 in (0, 1) and tb // 4 < NBLK:
                    emit_OP_half(tb // 4, tb % 4)

    nc.compile()
    return nc


_NC_CACHE = {}
LAST_RESULT = None


def make_in_maps(content, style, scale, shift, W1, b1, W2, b2, Wsc, bsc,
                 Wsh, bsh, Wso, bso, Wsho, bsho, pos_emb):
    inv = 1.0 / np.sqrt(EMB / HEADS)
    f32 = np.float32

    w1t = (np.asarray(W1, f32)[_RHO].T * inv).astype(_bf16)  # [e_in, e_out], scaled
    w2t = np.asarray(W2, f32)[_RHO].T.astype(_bf16)
    wsct = np.asarray(Wsc, f32)[_RHO].T.astype(_bf16)
    wsht = np.asarray(Wsh, f32)[_RHO].T.astype(_bf16)
    wsot = np.asarray(Wso, f32).T.astype(_bf16)
    wshot = np.asarray(Wsho, f32).T.astype(_bf16)
    b1r = (np.asarray(b1, f32)[_RHO] * inv).reshape(4, 128).T.copy()
    b2r = np.asarray(b2, f32)[_RHO].reshape(4, 128).T.copy()
    bso2 = np.asarray(Wso, f32) @ np.asarray(bsc, f32)[_RHO] + np.asarray(bso, f32)
    bsho2 = (np.asarray(Wsho, f32) @ np.asarray(bsh, f32)[_RHO]
             + np.asarray(bsho, f32))
    bsor = bso2.reshape(4, 128).T.copy()
    bshor = bsho2.reshape(4, 128).T.copy()
    ptab = _pair_tables(np.asarray(pos_emb, f32))
    ones2 = np.zeros((128, 128), dtype=_bf16)
    ones2[:64, :64] = 1
    ones2[64:, 64:] = 1

    common = dict(
        w1t=w1t, w2t=w2t, wsct=wsct, wsht=wsht, wsot=wsot, wshot=wshot,
        b1r=b1r, b2r=b2r, bsor=bsor, bshor=bshor, ptab=ptab, ones2=ones2,
    )
    in_maps = []
    for b in range(NCORES):
        m = dict(common)
        for name, full in (("cw", content), ("sw", style),
                           ("scw", scale), ("shw", shift)):
            x = np.asarray(full[b], f32)[_PERM]           # [N, EMB] window order
            m[name] = np.ascontiguousarray(x.T).astype(_bf16)
        in_maps.append(m)
    return in_maps


def kernel(**inputs):
    global LAST_RESULT
    from concourse.bass_utils import run_bass_kernel_spmd

    in_maps = make_in_maps(**inputs)

    if "nc" not in _NC_CACHE:
        _NC_CACHE["nc"] = _build_bass()
    res = run_bass_kernel_spmd(_NC_CACHE["nc"], in_maps, list(range(NCORES)))
    LAST_RESULT = res

    out_sc = np.empty((B, N, EMB), np.float32)
    out_sh = np.empty((B, N, EMB), np.float32)
    for b in range(NCORES):
        out_sc[b][_OPERM] = res.results[b]["yso"].T
        out_sh[b][_OPERM] = res.results[b]["ysho"].T
    return out_sc, out_sh
